# revision 71
# baseline (speedup 1.0000x reference)
"""Distributed multi-head-attention kernel for 8 TRN2 NeuronCores.

Problem (hardcoded): B=4, T=S=1024, E=512, H=8, head_dim=64, fp32 I/O.
Sharding: core c handles batch b=c//2 and heads [4*(c%2), 4*(c%2)+4).
No collectives: each core produces a partial output projection
(contraction over its 256 ctx columns); the host sums the two partials
per batch and adds bo.

Compute dtype: bf16 on the TensorEngine (fp32 PSUM accumulation),
softmax exp in fp32 on ScalarE. The attention inner loop is paced by
ScalarE exp ([128,1024] tiles); all PE work (projections, scores, ctx,
output projection) is scheduled inside/around it with statically placed
PSUM banks so the scores ping-pong never stalls on pool-ring parity.
"""

import numpy as np
import ml_dtypes

import concourse.bass as bass
import concourse.tile as tile
import concourse.mybir as mybir
from concourse.bass_utils import run_bass_kernel_spmd

BF16 = mybir.dt.bfloat16
F32 = mybir.dt.float32
F32R = mybir.dt.float32r
FP8 = mybir.dt.float8e3
NPBF16 = ml_dtypes.bfloat16
NPFP8 = ml_dtypes.float8_e3m4
# q/k projection inputs+weights travel as fp8 e3m4; weights are pre-scaled by
# WSCALE on the host so they sit in e3m4's normal range, and the 1/WSCALE^2 is
# folded into the softmax exp scale.
WSCALE = 64.0

B, T, S, E = 4, 1024, 1024, 512
H, HD = 8, 64
N_CORES = 8
HPC = H // 2          # heads per core = 4
OS = E // 2           # o-slice width per core = 256
KT = E // 128         # contraction k-tiles for projections = 4
TT = T // 128         # token tiles = 8
TC = T // 512         # 512-wide token chunks = 2
NIT = HPC * TT        # attention iterations = 32

# ---------------------------------------------------------------------------
# Walrus in this container rejects instructions carrying more than a couple of
# sync waits. After Tile scheduling, split excess waits onto same-engine NOPs
# inserted immediately before the over-subscribed instruction.
# ---------------------------------------------------------------------------
_MAX_WAITS = 1
_split_ctr = [0]


def _split_sync_waits(nc, max_waits=_MAX_WAITS):
    for f in nc.m.functions:
        for bb in f.blocks:
            insts = bb.instructions
            if not any(i.sync_info and i.sync_info.on_wait
                       and len(i.sync_info.on_wait) > max_waits for i in insts):
                continue
            new = []
            for inst in insts:
                si = inst.sync_info
                if si is not None and si.on_wait and len(si.on_wait) > max_waits:
                    waits = list(si.on_wait)
                    extra, keep = waits[:-max_waits], waits[-max_waits:]
                    for j in range(0, len(extra), max_waits):
                        _split_ctr[0] += 1
                        nop = mybir.InstNoOp(
                            name=f"syncsplit-{_split_ctr[0]}", ins=[], outs=[])
                        nop.engine = inst.engine
                        nop.bass_nofuse = True
                        nop.text_hint = "syncsplit"
                        nop.sync_info = mybir.SyncInfo(
                            on_wait=extra[j:j + max_waits], on_update=[])
                        new.append(nop)
                    si.on_wait = keep
                new.append(inst)
            bb.instructions = new


def _drain_and_barrier_light(self, tick_clock, wait_clock):
    # Exit path: a single Sync drain (waits for all DMA completions); skip the
    # all-engine barrier + semaphore clearing — the NEFF executes once per
    # process, so post-run semaphore state doesn't matter.
    from concourse.vector_clock import ScopedClock
    nc = self.nc
    drain_inst = nc.sync.drain()
    wait_clock.add_sem_waits(
        drain_inst.ins, ScopedClock({None: tick_clock.global_clock}))
    assert self.sems is not None
    popped = nc._tile_sem_poison_stack.pop()
    assert popped is self._sem_poison


tile.TileContext._drain_and_barrier = _drain_and_barrier_light


# ---------------------------------------------------------------------------
# Kernel graph
# ---------------------------------------------------------------------------
def build_nc():
    nc = bass.Bass()

    # p-major layouts: [p, k, n] flattened so DMAs are contiguous per
    # partition.  The q/k projection operands are packed into three tensors so
    # the critical prefix is three large DMA triggers:
    #   qpk[p, k, 0:OS]=wq, [OS:OS+512]=xq(t 0:512); kpk same for wk/xk;
    #   xb[p, k, 0:512]=xq(t 512:1024), [512:1024]=xk(t 512:1024).
    PW = OS + 512
    qpk = nc.declare_dram_parameter("qpk", [128, KT * PW], FP8, isOutput=False)
    kpk = nc.declare_dram_parameter("kpk", [128, KT * PW], FP8, isOutput=False)
    xb = nc.declare_dram_parameter("xb", [128, KT * T], FP8, isOutput=False)
    bqbk = nc.declare_dram_parameter("bqbk", [128, 4], F32, isOutput=False)
    wvT = nc.declare_dram_parameter("wvT", [128, KT * OS], BF16, isOutput=False)
    bv_b = nc.declare_dram_parameter("bv_b", [128, OS], F32, isOutput=False)
    xvT = nc.declare_dram_parameter("xvT", [128, KT * S], BF16, isOutput=False)
    # Wo^T head-pair slices: [128 (2 heads x 64 c), 512 (e)] each
    woPs = [nc.declare_dram_parameter(f"woP{p}", [128, E], BF16, isOutput=False)
            for p in range(2)]
    # out[p, tt, e] = output token tt*128+p, feature e (host un-permutes)
    out_ext = nc.declare_dram_parameter("out", [128, TT * E], BF16,
                                        isOutput=True)

    with tile.TileContext(nc) as tc:
        with (
            tc.tile_pool(name="inp", bufs=1) as inp,
            tc.tile_pool(name="wts", bufs=1) as wts,
            tc.tile_pool(name="act", bufs=1) as actp,
            tc.tile_pool(name="et", bufs=4) as etp,
            tc.tile_pool(name="rb", bufs=2) as rbp,
            tc.tile_pool(name="cr", bufs=2) as crp,
            tc.tile_pool(name="ob", bufs=2) as obp,
        ):
            # ---- static PSUM: 4 x [128, 1024] fp32 = 8 banks ----------------
            SE = nc.place_psum_tensor("SE", [128, T], F32, bank=0)
            SO = nc.place_psum_tensor("SO", [128, T], F32, bank=2)
            CE = nc.place_psum_tensor("CE", [128, T], F32, bank=4)
            CO = nc.place_psum_tensor("CO", [128, T], F32, bank=6)
            spsum = [SE, SO]
            cpsum = [CE, CO]

            # ---- input DMAs, ordered by first use ---------------------------
            qpk_sb = inp.tile([128, KT // 2, 2, PW], FP8, name="qpk")
            nc.sync.dma_start(
                qpk_sb[:], qpk.ap().rearrange("p (j o w) -> p j o w", j=2, o=2))
            kpk_sb = inp.tile([128, KT // 2, 2, PW], FP8, name="kpk")
            nc.sync.dma_start(
                kpk_sb[:], kpk.ap().rearrange("p (j o w) -> p j o w", j=2, o=2))
            xb_sb = inp.tile([128, KT // 2, 2, T], FP8, name="xb")
            nc.sync.dma_start(
                xb_sb[:], xb.ap().rearrange("p (j o t) -> p j o t", j=2, o=2))
            bqbk_sb = wts.tile([128, 4], F32, name="bqbk")
            nc.sync.dma_start(bqbk_sb[:], bqbk.ap())
            wv_sb = wts.tile([128, KT, OS], BF16, name="wv")
            nc.sync.dma_start(wv_sb[:], wvT.ap().rearrange("p (k o) -> p k o", k=KT))
            xv_sb = inp.tile([128, KT, S], BF16, name="xv")
            rrv = xvT.ap().rearrange("p (k t) -> p k t", k=KT)
            nc.sync.dma_start(xv_sb[:, :, 0:256], rrv[:, :, 0:256])
            bv_sb = wts.tile([128, OS], F32, name="bv")
            nc.sync.dma_start(bv_sb[:], bv_b.ap())
            for q in range(1, 4):
                nc.sync.dma_start(xv_sb[:, :, 256 * q:256 * (q + 1)],
                                  rrv[:, :, 256 * q:256 * (q + 1)])
            wo_sbs = []
            for p in range(2):
                wo_sb = wts.tile([128, E], BF16, name=f"wo{p}")
                nc.sync.dma_start(wo_sb[:], woPs[p].ap())
                wo_sbs.append(wo_sb)

            # ---- activations ------------------------------------------------
            qT_sb = [actp.tile([128, T], BF16, name=f"qT{ot}") for ot in range(2)]
            kT_sb = [actp.tile([128, S], BF16, name=f"kT{ot}") for ot in range(2)]
            v_aug = [actp.tile([128, HPC, HD + 1], BF16, name=f"vaug{st}")
                     for st in range(TT)]
            # paired normalized ctx: pair p holds heads 2p (rows 0-63) and
            # 2p+1 (rows 64-127), laid out [c, t]
            ctxp = [actp.tile([128, T], BF16, name=f"ctxp{p}") for p in range(2)]
            ones_sb = wts.tile([16, HD], F32, name="ones")
            nc.gpsimd.memset(ones_sb[:], 1.0)
            ones_hi = wts.tile([HD + 1, 1], F32, name="oneshi")
            nc.gpsimd.memset(ones_hi[:], 1.0)
            # Dummy activation so the ~1.5us ACT_TABLE_LOAD happens during the
            # DMA lead-in instead of right before the first real exp.
            warm_sb = wts.tile([1, HD], F32, name="actwarm")
            nc.scalar.activation(warm_sb[:], ones_sb[0:1, :],
                                 mybir.ActivationFunctionType.Exp)
            for st in range(TT):
                nc.gpsimd.memset(v_aug[st][:, :, HD:HD + 1], 1.0)
            # Tiny matmuls during the input-DMA lead-in: the PE HAM clock gate
            # needs ~3.4us of sustained activity before it unthrottles, so burn
            # the wait warming it up instead of running the projections cold.
            for w in range(32):
                nc.tensor.matmul(
                    SE.ap()[0:HD, 0:HD], ones_sb.bitcast(F32R),
                    ones_sb.bitcast(F32R), start=True, stop=True)

            def qk_quarter(which, ot, tc_i, ps):
                # ps: [128, 512] psum view
                pk, dst = ((qpk_sb, qT_sb), (kpk_sb, kT_sb))[which]
                for j in range(KT // 2):
                    for ko in range(2):
                        if tc_i == 0:
                            src = pk[:, j, ko, OS:OS + 512]
                        else:
                            src = xb_sb[:, j, ko, 512 * which:512 * (which + 1)]
                        nc.tensor.matmul(
                            ps,
                            pk[:, j, ko, 128 * ot:128 * (ot + 1)],
                            src,
                            start=(j == 0 and ko == 0),
                            stop=(j == KT // 2 - 1 and ko == 1),
                        )
                nc.vector.tensor_scalar_add(
                    dst[ot][:, 512 * tc_i:512 * (tc_i + 1)], ps,
                    bqbk_sb[:, 2 * which + ot:2 * which + ot + 1])

            def v_proj(st):
                # v_aug[st][p, h, 0:64] = v[s, 64h:64h+64] (+bv); col 64 = 1
                ps = CO.ap()[:, 256 * (st % 4):256 * (st % 4) + 256]
                for k in range(KT):
                    nc.tensor.matmul(
                        ps,
                        xv_sb[:, k, 128 * st:128 * (st + 1)],
                        wv_sb[:, k, :],
                        start=(k == 0), stop=(k == KT - 1),
                    )
                nc.vector.tensor_add(
                    v_aug[st][:, :, 0:HD],
                    ps.rearrange("p (h d) -> p h d", h=HPC),
                    bv_sb.rearrange("p (h d) -> p h d", h=HPC),
                )

            def scores(i):
                h, st = divmod(i, TT)
                ot, po = h // 2, HD * (h % 2)
                ps = spsum[i % 2]
                for tc_i in range(TC):
                    nc.tensor.matmul(
                        ps.ap()[:, 512 * tc_i:512 * (tc_i + 1)],
                        kT_sb[ot][po:po + HD, 128 * st:128 * (st + 1)],
                        qT_sb[ot][po:po + HD, 512 * tc_i:512 * (tc_i + 1)],
                        start=True, stop=True,
                    )

            def make_tail(h, copy_on_scalar=False):
                # Softmax denominators live in row HD of c_ps.  Stage 1 (now,
                # right after ctx(h,7)): copy just the denominator row to SBUF
                # so the transpose chain isn't gated on the big accumulator
                # copy.  Stage 2 (st2 of the next head): copy the ctx rows
                # (frees the bank), transpose the row partition-parallel with
                # 8 tiny PE transposes into the freed bank, reciprocal on 128
                # lanes, one reshape DMA (GpSimd software-DGE) back to a
                # [1, T] row.  Stage 3 (st6): broadcast with two K=1 f32r
                # matmuls and normalize+cast into the ctx pair tile.
                cps = cpsum[h % 2]
                p, row = h // 2, HD * (h % 2)
                cp = nc.scalar.copy if copy_on_scalar else nc.vector.tensor_copy
                craw = crp.tile([HD + 1, T], F32, name="craw")
                cp(craw[HD:HD + 1, :], cps.ap()[HD:HD + 1, :])

                def stage2():
                    cp(craw[0:HD, :], cps.ap()[0:HD, :])
                    rv = craw[HD:HD + 1, :].rearrange("p (i j) -> p j i", j=TT)
                    for jj in range(TT):
                        nc.tensor.matmul(
                            cps.ap()[:, jj:jj + 1],
                            rv[:, jj, :],
                            ones_hi[HD:HD + 1, 0:1],
                            is_transpose=True, start=True, stop=True)
                    rinvT = rbp.tile([128, TT], F32, name="rinvT")
                    nc.vector.reciprocal(rinvT[:], cps.ap()[:, 0:TT])
                    rinv = rbp.tile([1, T], F32, name="rinv")
                    nc.gpsimd.dma_start(rinv[:], rinvT[:])

                    def tail_b():
                        for half in range(2):
                            sl = slice(512 * half, 512 * (half + 1))
                            nc.tensor.matmul(
                                cps.ap()[0:HD, sl],
                                ones_sb[0:1, :].bitcast(F32R),
                                rinv[0:1, sl].bitcast(F32R),
                                start=True, stop=True)
                        for half in range(2):
                            sl = slice(512 * half, 512 * (half + 1))
                            nc.vector.tensor_mul(
                                ctxp[p][row:row + HD, sl],
                                craw[0:HD, sl],
                                cps.ap()[0:HD, sl],
                            )
                    return tail_b
                return stage2

            # ---- pre-attention: all q/k projection quarters ----------------
            # First the three quarters scores(0) needs, then scores(0), then
            # the rest into the (still free) ctx banks.
            qk_quarter(0, 0, 0, SE.ap()[:, 0:512])
            qk_quarter(0, 0, 1, SE.ap()[:, 512:1024])
            qk_quarter(1, 0, 0, SO.ap()[:, 0:512])
            scores(0)
            qk_quarter(1, 0, 1, SO.ap()[:, 512:1024])
            qk_quarter(0, 1, 0, CE.ap()[:, 0:512])
            qk_quarter(1, 1, 0, CE.ap()[:, 512:1024])
            qk_quarter(0, 1, 1, CO.ap()[:, 0:512])
            qk_quarter(1, 1, 1, CO.ap()[:, 512:1024])

            # ---- attention loop, paced by ScalarE exp ----------------------
            exp_scale = float(1.0 / (np.sqrt(HD) * WSCALE * WSCALE))
            pending_tail = None
            pending_a2 = None
            pending_b = None
            for i in range(NIT):
                h, st = divmod(i, TT)
                if h == 0:
                    v_proj(st)
                if i + 1 < NIT:
                    scores(i + 1)
                et_t = etp.tile([128, T], BF16, name="et")
                nc.scalar.activation(
                    et_t[:], spsum[i % 2].ap()[:, :],
                    mybir.ActivationFunctionType.Exp,
                    scale=exp_scale)
                cps = cpsum[h % 2]
                for tc_i in range(TC):
                    nc.tensor.matmul(
                        cps.ap()[0:HD + 1, 512 * tc_i:512 * (tc_i + 1)],
                        v_aug[st][:, h, :],
                        et_t[:, 512 * tc_i:512 * (tc_i + 1)],
                        start=(st == 0), stop=(st == TT - 1),
                    )
                if st == 2 and pending_a2 is not None:
                    pending_b = pending_a2()
                    pending_a2 = None
                if st == 6 and pending_b is not None:
                    pending_b()
                    pending_b = None
                if st == TT - 1:
                    # denominator-row copy fires immediately after ctx(h,7)
                    pending_a2 = make_tail(
                        h, copy_on_scalar=(h == HPC - 1))

            # ---- output projection: K=128 head pairs -----------------------
            # Pair-0 (heads 0,1) partials for groups 0-2 go first so the PE is
            # busy while head 3's denominator chain runs; group 3's bank (CO)
            # is still owned by that chain, so its pair-0 comes after.
            o_banks = [SE, SO, CE, CO]

            def oproj(g, p):
                for half in range(2):
                    tt = 2 * g + half
                    nc.tensor.matmul(
                        o_banks[g].ap()[:, 512 * half:512 * (half + 1)],
                        ctxp[p][:, 128 * tt:128 * (tt + 1)],
                        wo_sbs[p][:],
                        start=(p == 0), stop=(p == 1),
                    )

            oproj(0, 0)
            oproj(1, 0)

            def warm_mm(n):
                # Dummy matmuls into CE keep the PE clock warm while head 3's
                # denominator chain runs; oproj(2,0)'s start=True wipes them.
                for w in range(n):
                    nc.tensor.matmul(
                        CE.ap()[0:HD, 0:512], wo_sbs[0][0:16, 0:HD],
                        wo_sbs[0][0:16, :], start=True, stop=True)

            warm_mm(4)
            tb3 = pending_a2()       # ctx copy, transposes, recip, reshape DMA
            warm_mm(6)
            oproj(2, 0)
            tb3()                    # broadcast + normalize

            def out_group(g):
                oproj(g, 1)
                ob = obp.tile([128, T], BF16, name="ob")
                if g % 2 == 0:
                    nc.scalar.copy(ob[:], o_banks[g].ap()[:, :])
                else:
                    nc.vector.tensor_copy(ob[:], o_banks[g].ap()[:, :])
                nc.sync.dma_start(
                    out_ext.ap().rearrange("p (tt e) -> p tt e", tt=TT)
                    [:, 2 * g:2 * g + 2, :],
                    ob.rearrange("p (tt e) -> p tt e", tt=2))

            out_group(0)
            out_group(1)
            oproj(3, 0)
            out_group(2)
            out_group(3)

    _split_sync_waits(nc)
    return nc


_NC = None


def _get_nc():
    global _NC
    if _NC is None:
        _NC = build_nc()
    return _NC


# ---------------------------------------------------------------------------
# Host-side sharding / unsharding
# ---------------------------------------------------------------------------
def make_in_maps(queries, keys, values, Wq, bq, Wk, bk, Wv, bv, Wo):
    in_maps = []
    for c in range(N_CORES):
        b, hh = divmod(c, 2)
        osl = slice(OS * hh, OS * (hh + 1))
        bqbk_s = np.zeros((128, 4), np.float32)
        bqbk_s[:, 0] = bq[osl][0:128] * WSCALE
        bqbk_s[:, 1] = bq[osl][128:256] * WSCALE
        bqbk_s[:, 2] = bk[osl][0:128] * WSCALE
        bqbk_s[:, 3] = bk[osl][128:256] * WSCALE

        def pmaj(a):
            # [E, N] -> [128, KT*N], k-tiles along the free axis
            e, n = a.shape
            return np.ascontiguousarray(
                a.reshape(KT, 128, n).transpose(1, 0, 2).reshape(128, KT * n))

        def pk2(a):
            # [E, N] -> [128, j, ko, N]: partition p, DoubleRow pair-tile j,
            # pair element ko holds contraction row (2j+ko)*128 + p
            e, n = a.shape
            return np.ascontiguousarray(
                a.reshape(KT // 2, 2, 128, n).transpose(2, 0, 1, 3))

        xq8 = pk2(queries[b].T).astype(NPFP8)
        xk8 = pk2(keys[b].T).astype(NPFP8)
        wq8 = pk2(Wq[osl, :].T * WSCALE).astype(NPFP8)
        wk8 = pk2(Wk[osl, :].T * WSCALE).astype(NPFP8)
        qpk = np.concatenate([wq8, xq8[:, :, :, 0:512]], axis=3)
        kpk = np.concatenate([wk8, xk8[:, :, :, 0:512]], axis=3)
        xb = np.concatenate(
            [xq8[:, :, :, 512:1024], xk8[:, :, :, 512:1024]], axis=3)
        m = {
            "qpk": np.ascontiguousarray(qpk.reshape(128, -1)),
            "kpk": np.ascontiguousarray(kpk.reshape(128, -1)),
            "xb": np.ascontiguousarray(xb.reshape(128, -1)),
            "xvT": pmaj(values[b].T).astype(NPBF16),
            "wvT": pmaj(Wv[osl, :].T).astype(NPBF16),
            "bqbk": bqbk_s,
            "bv_b": np.broadcast_to(
                bv[osl][None, :], (128, OS)).astype(np.float32).copy(),
        }
        for p in range(2):
            cs = slice(OS * hh + 128 * p, OS * hh + 128 * (p + 1))
            m[f"woP{p}"] = np.ascontiguousarray(Wo[:, cs].T).astype(NPBF16)
        in_maps.append(m)
    return in_maps


def run_device(in_maps, trace=False):
    nc = _get_nc()
    return run_bass_kernel_spmd(
        nc, in_maps, core_ids=list(range(N_CORES)), trace=trace)


def _numpy_reference(queries, keys, values, Wq, bq, Wk, bk, Wv, bv, Wo, bo,
                     q_padding_mask, key_padding_mask, attn_mask):
    q = queries @ Wq.T + bq
    k = keys @ Wk.T + bk
    v = values @ Wv.T + bv

    def split(x):
        b, l, e = x.shape
        return x.reshape(b, l, H, HD).transpose(0, 2, 1, 3)

    q, k, v = split(q), split(k), split(v)
    scores = np.einsum('bhtd,bhsd->bhts', q, k) / np.sqrt(HD)
    scores = np.where(key_padding_mask[:, None, None, :], -np.inf, scores)
    scores = np.where(~attn_mask[None, None, :, :], -np.inf, scores)
    scores = scores - scores.max(axis=-1, keepdims=True)
    w = np.exp(scores)
    w = w / w.sum(axis=-1, keepdims=True)
    w = np.where(q_padding_mask[:, None, :, None], 0.0, w)
    ctx = np.einsum('bhts,bhsd->bhtd', w, v)
    ctx = ctx.transpose(0, 2, 1, 3).reshape(queries.shape[0], -1, E)
    return (ctx @ Wo.T + bo).astype(np.float32)


def kernel(queries, keys, values, Wq, bq, Wk, bk, Wv, bv, Wo, bo,
           q_padding_mask, key_padding_mask, attn_mask):
    queries = np.asarray(queries, dtype=np.float32)
    keys = np.asarray(keys, dtype=np.float32)
    values = np.asarray(values, dtype=np.float32)
    Wq, bq = np.asarray(Wq, np.float32), np.asarray(bq, np.float32)
    Wk, bk = np.asarray(Wk, np.float32), np.asarray(bk, np.float32)
    Wv, bv = np.asarray(Wv, np.float32), np.asarray(bv, np.float32)
    Wo, bo = np.asarray(Wo, np.float32), np.asarray(bo, np.float32)
    q_padding_mask = np.asarray(q_padding_mask)
    key_padding_mask = np.asarray(key_padding_mask)
    attn_mask = np.asarray(attn_mask)

    # The device kernel skips masking (and softmax max-subtraction, valid for
    # this problem's bounded score range). Masks are all-trivial per the
    # problem spec; fall back to a host reference if they ever are not.
    if q_padding_mask.any() or key_padding_mask.any() or not attn_mask.all():
        return _numpy_reference(
            queries, keys, values, Wq, bq, Wk, bk, Wv, bv, Wo, bo,
            q_padding_mask, key_padding_mask, attn_mask)

    in_maps = make_in_maps(queries, keys, values, Wq, bq, Wk, bk, Wv, bv, Wo)
    res = run_device(in_maps, trace=False)
    out = np.empty((B, T, E), np.float32)
    for b in range(B):
        # device layout: [p, tt, e] -> token tt*128+p
        o0 = res.results[2 * b]["out"].astype(np.float32)
        o1 = res.results[2 * b + 1]["out"].astype(np.float32)
        o = (o0 + o1).reshape(128, TT, E).transpose(1, 0, 2).reshape(T, E)
        out[b] = o + bo[None, :]
    return out
